# revision 25
# baseline (speedup 1.0000x reference)
"""Self-contained Trainium2 kernel for nn_B3SplineUWT (3-level B3-spline
undecimated wavelet transform), data-parallel over 8 NeuronCores.

kernel(x: [8,1024,1024] f32) -> [8,4,1024,1024] f32  (w1,w2,w3,c3)

Per core: one image, bf16 datapath (tolerance 2e-2; bf16 lands ~6e-3).
  - H-conv (partition dim): PE banded matmuls; taps pre-scaled by 1/16
    (exact in bf16) so the W-conv uses unnormalized integer taps.
  - W-conv (free dim), per 512-half routable:
      route PE : 5 accumulating shift-matmuls (integer-scaled identity
                 stationaries, shifted moving APs, no transposes)
      route DVE: [1,4,6,4,1] = [1,1]^4 -> four shifted tensor_adds
                 (all 2x bf16 mode) over a reflect-margined tile
  - Subtracts w_j = c_{j-1} - c_j fused per 2-chunk pair on DVE.
  - All HBM I/O via SWDGE cast-DMAs (f32<->bf16 in the DMA): quarter
    input loads, pair-granular output stores, chunk-granular final
    stores (short tail drain); PSUM split into H/W pools; dummy PE
    warm-up matmuls ramp the clock during the input DMAs.
"""
import numpy as np

import concourse.bacc as bacc
import concourse.bass as bass
import concourse.mybir as mybir
import concourse.tile as tile
from concourse.bass_utils import run_bass_kernel_spmd

F32 = mybir.dt.float32
BF16 = mybir.dt.bfloat16
ADD = mybir.AluOpType.add
MULT = mybir.AluOpType.mult

B = 8
H = 1024
W = 1024
P = 128
NCH = H // P
LEVELS = 3
DILS = (1, 2, 4)
MARG = 8
WE = W + 2 * MARG

# H-conv taps with the 1/16 W-normalization folded in (exact in bf16)
HTAPS = {0: 3.0 / 128, 1: 1.0 / 64, 2: 1.0 / 256}
# W-conv integer taps (route PE)
WTAPS = {0: 6.0, 1: 4.0, 2: 1.0}

# ---- tunable schedule config ----
CFG = {
    # per level: {chunk: n PE halves (0..2)}; missing = 0 (all DVE)
    "npe": {0: {1: 2, 4: 2, 6: 1}, 1: {1: 2, 4: 2, 6: 1},
            2: {1: 2, 4: 2, 6: 1}},
    # (level, chunk) -> engine for the first binomial add: "dve" | "pool"
    "a1": {},
    "warmup": 8,
    "tailsplit": 2,
    "split_psum": True,
    "subpool": (),
    "yhp": 4,
    "wtm": 6,
}


def _reflect(i, n):
    if i < 0:
        return -i
    if i >= n:
        return 2 * (n - 1) - i
    return i


def _build_h_bands():
    out = []
    for j, d in enumerate(DILS):
        full = np.zeros((H, H), np.float64)
        for r in range(H):
            for o in (-2 * d, -d, 0, d, 2 * d):
                full[_reflect(r + o, H), r] += HTAPS[abs(o) // d]
        blocks = {}
        for co in range(NCH):
            for ci in range(NCH):
                blk = full[ci * P:(ci + 1) * P, co * P:(co + 1) * P]
                if np.any(blk != 0):
                    blocks[(co, ci)] = np.ascontiguousarray(
                        blk.astype(np.float32))
        out.append(blocks)
    return out


def _pack_consts(h_bands):
    """Pack level-0 bands + identities first so a small head DMA can
    unblock the first chunks; returns (packed, index, wid_off, n0)."""
    mats, seen = [], {}
    index = []
    wid_off = {}

    def add_level(blocks):
        idx = {}
        for key in sorted(blocks):
            b = blocks[key]
            hsh = b.tobytes()
            if hsh not in seen:
                seen[hsh] = len(mats) * P
                mats.append(b)
            idx[key] = seen[hsh]
        return idx

    index.append(add_level(h_bands[0]))
    for t, s in WTAPS.items():
        wid_off[t] = len(mats) * P
        mats.append(np.eye(P, dtype=np.float32) * s)
    n0 = len(mats) * P
    for blocks in h_bands[1:]:
        index.append(add_level(blocks))
    packed = np.ascontiguousarray(
        np.concatenate(mats, axis=1).astype(np.float32))
    return packed, index, wid_off, n0


def _build_program(cfg=None):
    cfg = cfg or CFG
    h_bands = _build_h_bands()
    consts_np, cindex, wid_off, ncols0 = _pack_consts(h_bands)
    ncols_const = consts_np.shape[1]

    nc = bacc.Bacc("TRN2", target_bir_lowering=False, debug=False)
    x_d = nc.dram_tensor("x", [H, W], F32, kind="ExternalInput")
    c_d = nc.dram_tensor("consts", [P, ncols_const], F32,
                         kind="ExternalInput")
    out_d = nc.dram_tensor("out", [LEVELS + 1, H, W], F32,
                           kind="ExternalOutput")

    splitps = cfg.get("split_psum", False)
    with tile.TileContext(nc) as tc:
        with tc.tile_pool(name="sb", bufs=1) as sb, \
             tc.tile_pool(name="yhp", bufs=cfg.get("yhp", 4)) as yhp, \
             tc.tile_pool(name="wtm", bufs=cfg.get("wtm", 6)) as wtm, \
             tc.tile_pool(name="wst", bufs=3) as wstage, \
             tc.tile_pool(name="ps", bufs=(3 if splitps else 4),
                          space="PSUM") as ps, \
             tc.tile_pool(name="psw", bufs=1, space="PSUM") as psw:

            cr = sb.tile([P, ncols_const], BF16, tag="cr", name="cr")
            # level-0 consts first (small, unblocks the first chunks)
            nc.gpsimd.dma_start(cr[:, 0:ncols0], c_d[:, 0:ncols0])

            # ---- PE warm-up: dummy matmuls ramp the clock while the
            # input DMAs are in flight ----
            if cfg.get("warmup"):
                wu = sb.tile([P, 512], BF16, tag="wu", name="wu")
                nc.vector.memset(wu[:], 0)
                pwu = ps.tile([P, 512], F32, tag="psum", name="pwu",
                              bufs=(3 if splitps else 4))
                for _ in range(cfg["warmup"]):
                    nc.tensor.matmul(pwu[:], wu[:, 0:P], wu[:],
                                     start=True, stop=True)

            xr = sb.tile([P, NCH, W], BF16, tag="xr", name="xr")
            cnr = [sb.tile([P, NCH, W], BF16, tag=f"cnr{i}", name=f"cnr{i}")
                   for i in range(2)]

            # ---- load x: 4 quarter cast-DMAs (f32 HBM -> bf16 SBUF),
            # remaining consts after all quarters (needed from level 1) ----
            for q in range(4):
                nc.gpsimd.dma_start(
                    xr[:, 2 * q:2 * q + 2, :],
                    bass.AP(x_d, q * 2 * P * W,
                            [[W, P], [P * W, 2], [1, W]]))
            nc.gpsimd.dma_start(cr[:, ncols0:], c_d[:, ncols0:])

            for j in range(LEVELS):
                d = DILS[j]
                prev = xr if j == 0 else cnr[(j - 1) % 2]
                cur = cnr[j % 2]
                npe_map = cfg["npe"].get(j, {})

                for c in range(NCH):
                    # ---- H-conv: banded matmuls on PE ----
                    pairs = sorted((key, off)
                                   for key, off in cindex[j].items()
                                   if key[0] == c)
                    ph = ps.tile([P, W], F32, tag="psum", name="ph",
                                 bufs=(3 if splitps else 4))
                    for g in range(2):
                        for i, ((_, ci), off) in enumerate(pairs):
                            nc.tensor.matmul(
                                ph[:, g * 512:(g + 1) * 512],
                                cr[:, off:off + P],
                                prev[:, ci, g * 512:(g + 1) * 512],
                                start=(i == 0),
                                stop=(i == len(pairs) - 1))

                    # ---- evac H into margined tile + reflect margins ----
                    yh = yhp.tile([P, WE], BF16, tag="yh", name="yh")
                    nc.scalar.copy(yh[:, MARG:MARG + W], ph[:])
                    meng = (nc.scalar
                            if (cfg.get("marg") == "act"
                                or (j, c) in cfg.get("margact", ()))
                            else nc.gpsimd)
                    mcopy = (meng.copy if meng is nc.scalar
                             else meng.tensor_copy)
                    mcopy(
                        bass.AP(yh.tensor, 0, [[WE, P], [1, MARG]]),
                        bass.AP(yh.tensor, 2 * MARG, [[WE, P], [-1, MARG]]))
                    mcopy(
                        bass.AP(yh.tensor, MARG + W, [[WE, P], [1, MARG]]),
                        bass.AP(yh.tensor, MARG + W - 2,
                                [[WE, P], [-1, MARG]]))

                    def yap(off, ln, yh=yh):
                        return bass.AP(yh.tensor, MARG + off, [[WE, P],
                                                               [1, ln]])

                    npe = npe_map.get(c, 0)
                    if npe:
                        # ---- W-conv route PE: 5 shift-matmuls/half ----
                        if splitps:
                            pw = psw.tile([P, npe * 512], F32, tag="psw",
                                          name="pw", bufs=1)
                        else:
                            pw = ps.tile([P, npe * 512], F32, tag="psum",
                                         name="pw", bufs=4)
                        offs = ((0, 0), (1, -d), (1, d), (2, -2 * d),
                                (2, 2 * d))
                        for g in range(npe):
                            for i, (t, o) in enumerate(offs):
                                nc.tensor.matmul(
                                    pw[:, g * 512:(g + 1) * 512],
                                    cr[:, wid_off[t]:wid_off[t] + P],
                                    yap(g * 512 + o, 512),
                                    start=(i == 0),
                                    stop=(i == len(offs) - 1))
                        nc.scalar.copy(cur[:, c, 0:npe * 512],
                                       pw[:, 0:npe * 512])

                    if npe < 2:
                        # ---- W-conv route DVE: [1,4,6,4,1] = [1,1]^4,
                        # four shifted adds ----
                        a1_pool = cfg["a1"].get((j, c)) == "pool"
                        base = npe * 512
                        wlen = W - base
                        u1 = wtm.tile([P, WE], BF16, tag="u1", name="u1")
                        u2 = wtm.tile([P, WE], BF16, tag="u2", name="u2")

                        def uap(t_, off, ln):
                            return bass.AP(t_.tensor, MARG + off,
                                           [[WE, P], [1, ln]])

                        # u1[t] = y[t] + y[t+d]      t in [base-2d, +wlen+d)
                        eng = nc.gpsimd if a1_pool else nc.vector
                        eng.tensor_add(uap(u1, base - 2 * d, wlen + 3 * d),
                                       yap(base - 2 * d, wlen + 3 * d),
                                       yap(base - d, wlen + 3 * d))
                        # u2[t] = u1[t-d] + u1[t]    t in [base-d, +wlen+d)
                        nc.vector.tensor_add(
                            uap(u2, base - d, wlen + 2 * d),
                            uap(u1, base - 2 * d, wlen + 2 * d),
                            uap(u1, base - d, wlen + 2 * d))
                        # u1[t] = u2[t] + u2[t+d]    t in [base-d, +wlen)
                        nc.vector.tensor_add(
                            uap(u1, base - d, wlen + d),
                            uap(u2, base - d, wlen + d),
                            uap(u2, base, wlen + d))
                        # cur[t] = u1[t-d] + u1[t]   t in [base, +wlen)
                        nc.vector.tensor_add(cur[:, c, base:W],
                                             uap(u1, base - d, wlen),
                                             uap(u1, base, wlen))

                    # ---- subtract + cast DMA out ----
                    last = j == LEVELS - 1
                    # fine-set start must align to pair boundary so no
                    # even chunk is left without a DMA path
                    ts = cfg.get("tailsplit", 0)
                    fine = ((last and ts and c >= (NCH - ts) // 2 * 2)
                            or cfg.get("wgran") == 1)
                    if fine:
                        # chunk-granular tail: shorter final DMA drain
                        wp = wstage.tile([P, 2, W], BF16, tag="wp",
                                         name="wp")
                        nc.vector.tensor_sub(wp[:, 0, :], prev[:, c, :],
                                             cur[:, c, :])
                        if last:
                            nc.gpsimd.dma_start(
                                bass.AP(out_d, 3 * H * W + c * P * W,
                                        [[W, P], [1, W]]),
                                cur[:, c, :])
                        nc.gpsimd.dma_start(
                            bass.AP(out_d, j * H * W + c * P * W,
                                    [[W, P], [1, W]]),
                            wp[:, 0, :])
                    elif c % 2 == 1:
                        c0 = c - 1
                        wp = wstage.tile([P, 2, W], BF16, tag="wp",
                                         name="wp")
                        sub_eng = (nc.gpsimd
                                   if (j, c0 // 2) in cfg.get("subpool", ())
                                   else nc.vector)
                        sub_eng.tensor_sub(wp[:], prev[:, c0:c0 + 2, :],
                                           cur[:, c0:c0 + 2, :])
                        if last:
                            nc.gpsimd.dma_start(
                                bass.AP(out_d, 3 * H * W + c0 * P * W,
                                        [[W, P], [P * W, 2], [1, W]]),
                                cur[:, c0:c0 + 2, :])
                        nc.gpsimd.dma_start(
                            bass.AP(out_d, j * H * W + c0 * P * W,
                                    [[W, P], [P * W, 2], [1, W]]),
                            wp[:])

    nc.compile()
    return nc, consts_np


_CACHE = {}


def _get_program():
    if "prog" not in _CACHE:
        _CACHE["prog"] = _build_program()
    return _CACHE["prog"]


def kernel(x, _trace=False, _trace_kwargs=None):
    """x: [8, 1024, 1024] float32 -> [8, 4, 1024, 1024] float32."""
    x = np.asarray(x)
    assert x.shape == (B, H, W) and x.dtype == np.float32
    nc, consts_np = _get_program()
    in_maps = [{"x": np.ascontiguousarray(x[b]), "consts": consts_np}
               for b in range(B)]
    kw = {}
    if _trace:
        kw = dict(trace=True, **(_trace_kwargs or {}))
    res = run_bass_kernel_spmd(nc, in_maps, core_ids=list(range(B)), **kw)
    out = np.stack([r["out"] for r in res.results], axis=0)
    if _trace:
        return out, res
    return out
